# revision 26
# baseline (speedup 1.0000x reference)
"""Trainium2 Bass kernel for 2-layer BiLSTM + classifier (nn_BiLSTM_45234595561814).

Strategy (8 NeuronCores, single SPMD launch, no collectives):
  - Each core q owns a 64-token window of T=512, FULL batch (B=64), split into
    NU=2 sub-windows of SW=32 tokens.  The two sub-windows' forward
    recurrences run as ONE lockstep "super-chain" (and both backwards as
    another): they are mutually independent, so lockstep costs no latency,
    but every engine instruction doubles its payload (gates ACT [128,512]),
    amortizing the ~352-cycle ACT pipe fill, and the hh matmuls share
    weights across sub-windows (4 matmuls of N=128 per super-step).
  - Sequence parallelism via truncated warmup: LSTM state decays ~0.5/step,
    so a chain zero-initialized WARM steps before its window converges to
    the exact state.  L0 chains span [sub-window-WARM, +SW+WARM) so L1
    warmups are fed locally; no cross-core exchange anywhere.
  - SPAN-SLOT storage: xaug/ctl/y0/y1/xg1 are all indexed by token position
    (span-slot), not by chain step.  One xaug serves both directions (B
    just reads blocks descending), and the L1 projection + classifier reads
    are contiguous N=512 matmuls.
  - One-tanh trick: i,f,o weight rows pre-scaled by 0.5 so sigmoid(z) =
    0.5*(1+tanh(z/2)) needs only tanh -> ONE ACT op for all 4 gates.
    State kept doubled (C=2c, hh=2h); cell update is 3 STT DVE ops + 1 for
    hh.  Whh pre-scaled by extra 0.5 to absorb hh=2h.  Cell state in bf16.
  - L0 input projection fused into the per-step PSUM accumulation (xaug and
    wihT0 zero-padded to K=128 for the FWL fast path).  L1 projection
    precomputed into SBUF (bf16, gate-major span-slot blocks) and injected
    into gate PSUM via bf16 identity-matmul; projection chunks are paced
    into L0-tail / L1 PE idle slots by readiness/deadline order.
  - Pad tokens (outside [0,512)) handled exactly: x/ones rows zero keep
    state at 0 through leading pads; an L1 control row drives the i-gate
    preact to -30000 on pad tokens so pad xg1 cannot perturb state.
  - Classifier interleaved into the L1 loop chunk-by-chunk as both
    directions' span-slots complete; final GEMM emitted transposed (tokens
    on partitions), tanh batched 4 span-blocks per ACT.

kernel(**inputs) takes the FULL inputs and returns the FULL [64,512,64] f32
output.  Self-contained: hardcodes all shapes; no sibling imports.
"""

import os

import numpy as np
import ml_dtypes

import concourse.bass as bass
import concourse.mybir as mybir
import concourse.tile as tile
from concourse import bacc
from concourse.bass_utils import run_bass_kernel_spmd

bf16 = ml_dtypes.bfloat16
F32, BF16 = mybir.dt.float32, mybir.dt.bfloat16
AluOp = mybir.AluOpType
ACT_TANH = mybir.ActivationFunctionType.Tanh
ACT_RELU = mybir.ActivationFunctionType.Relu

H = 128          # rnn size
B = 64           # batch
T = 512          # seq len
D = 64           # input size
NC = 8           # cores
WIN = T // NC    # tokens per core window = 64
NU = 2           # sub-windows per core
SW = WIN // NU   # sub-window size = 32
B2 = NU * B      # columns per super-slot = 128
WARM = int(os.environ.get("BILSTM_WARM", "8"))
STATE_BF16 = os.environ.get("BILSTM_STATE_BF16", "1") == "1"
S0 = SW + 2 * WARM   # L0 super-chain steps = 48
S1 = SW + WARM       # L1 super-chain steps = 40
PADKILL = -30000.0
KP = 128         # padded contraction dim for L0 inproj (D+1 -> 128, FWL)
NTOK = SW * B2   # classifier columns = 4096

_CACHE = {}


def _build_program():
    nc = bacc.Bacc(None, target_bir_lowering=False)

    # ---------------- I/O declarations ----------------
    ei = lambda name, shape, dt=BF16: nc.dram_tensor(name, shape, dt, kind="ExternalInput")
    xaug = ei("xaug", [KP, S0 * B2])   # span-slot blocks; rows 0..63 x.T, row 64 ones
    ctl = ei("ctl", [2, S0 * B2])      # row0 valid, row1 padkill indicator
    wihT0 = {d: ei(f"wihT0{d}", [KP, 4 * H]) for d in "fb"}
    whhT0 = {d: ei(f"whhT0{d}", [H, 4 * H]) for d in "fb"}
    whhT1 = {d: ei(f"whhT1{d}", [H, 4 * H]) for d in "fb"}
    wih1Ta = {d: ei(f"wih1Ta{d}", [H, 4 * H]) for d in "fb"}   # y0F K-tile
    wih1Tb = {d: ei(f"wih1Tb{d}", [H, 4 * H]) for d in "fb"}   # y0B K-tile
    ctlT1 = {d: ei(f"ctlT1{d}", [2, 4 * H]) for d in "fb"}     # padkill row (row0 zero)
    biasg = {d: ei(f"biasg{d}", [H, 4], F32) for d in "fb"}    # L1 bias per gate col
    b1col = ei("b1col", [H, 2], F32)                           # cls bias per m-half
    idn = ei("idn", [H, H])
    w1Ta = ei("w1Ta", [H, 2 * H])   # (0.5*W1).T rows 0:128  -> [128, 256]
    w1Tb = ei("w1Tb", [H, 2 * H])   # rows 128:256
    w2Ta = ei("w2Ta", [H, D])       # W2.T rows 0:128 -> [128, 64]
    w2Tb = ei("w2Tb", [H, D])
    b2row = ei("b2row", [1, D])
    out = nc.dram_tensor("out", [NTOK, D], F32, kind="ExternalOutput")

    SDT = BF16 if STATE_BF16 else F32

    with tile.TileContext(nc) as tc:
        with tc.tile_pool(name="singles", bufs=1) as singles, \
             tc.tile_pool(name="state", bufs=1) as state, \
             tc.tile_pool(name="tpool", bufs=4) as tpool, \
             tc.tile_pool(name="vpool", bufs=3) as vpool, \
             tc.tile_pool(name="clssb", bufs=3) as clssb, \
             tc.tile_pool(name="psA", bufs=3, space="PSUM") as psA, \
             tc.tile_pool(name="psB", bufs=3, space="PSUM") as psB, \
             tc.tile_pool(name="psP", bufs=2, space="PSUM") as psP:

            # ---------------- load constants ----------------
            # weight loads issue from the ACT queue (idle at startup) so they
            # don't serialize behind the xaug chunks on the SP queue
            def load(src, shape, dt=BF16, eng=None):
                t = singles.tile(shape, dt, name=src.name, tag=src.name)
                (eng or nc.scalar).dma_start(out=t[:], in_=src[:])
                return t

            # xaug split into column chunks; F reads blocks ascending
            # (chunk 0 first), B descending (chunk 3 first)
            xaug_t = singles.tile([KP, S0 * B2], BF16, name="xaug", tag="xaug")
            XCH = S0 * B2 // 4
            for i in (0, 3):
                nc.sync.dma_start(out=xaug_t[:, i * XCH:(i + 1) * XCH],
                                  in_=xaug[:, i * XCH:(i + 1) * XCH])
            wihT0_t = {d: load(wihT0[d], [KP, 4 * H]) for d in "fb"}
            whhT0_t = {d: load(whhT0[d], [H, 4 * H]) for d in "fb"}
            for i in (1, 2):
                nc.sync.dma_start(out=xaug_t[:, i * XCH:(i + 1) * XCH],
                                  in_=xaug[:, i * XCH:(i + 1) * XCH])
            ctl_t = load(ctl, [2, S0 * B2])
            whhT1_t = {d: load(whhT1[d], [H, 4 * H]) for d in "fb"}
            wih1Ta_t = {d: load(wih1Ta[d], [H, 4 * H]) for d in "fb"}
            wih1Tb_t = {d: load(wih1Tb[d], [H, 4 * H]) for d in "fb"}
            ctlT1_t = {d: load(ctlT1[d], [2, 4 * H]) for d in "fb"}
            biasg_t = {d: load(biasg[d], [H, 4], F32) for d in "fb"}
            b1col_t = load(b1col, [H, 2], F32)
            idn_t = load(idn, [H, H])
            w1Ta_t = load(w1Ta, [H, 2 * H])
            w1Tb_t = load(w1Tb, [H, 2 * H])
            w2Ta_t = load(w2Ta, [H, D])
            w2Tb_t = load(w2Tb, [H, D])
            b2row_t = load(b2row, [1, D])

            # ---------------- persistent state (span-slot layouts) ----------
            y0 = {d: state.tile([H, S0 * B2], BF16, name=f"y0{d}", tag=f"y0{d}") for d in "fb"}
            y1 = {d: state.tile([H, S1 * B2], BF16, name=f"y1{d}", tag=f"y1{d}") for d in "fb"}
            h00 = state.tile([H, B2], BF16, name="h00", tag="h00")
            nc.vector.memset(h00[:], 0.0)
            # L1 projection, bf16, span-slot blocks of [4 gates x B2]
            xg1 = {d: state.tile([H, S1 * 4 * B2], BF16, name=f"xg1{d}", tag=f"xg1{d}") for d in "fb"}

            # span-slot of chain d at step s (L0 / L1)
            sp0 = lambda d, s: s if d == "f" else S0 - 1 - s
            sp1 = lambda d, s: s if d == "f" else S1 - 1 - s

            # ---------------- super-step primitives ----------------
            # gate PSUM tile: [H, 4*B2] f32 = exactly 1 bank, col = g*B2+u*B+b
            # JOINT T tile [H, 2*5*B2]: per-dir halves [o|i|f|g|C]; the two
            # dirs' C regions are a strided view so tanh(c) is ONE ACT instr
            # for both directions (the only cross-dir join: symmetric, so it
            # costs no latency but halves the ACT fixed overhead).
            DI = {"f": 0, "b": 1}
            pend = {}     # (layer, d, step) -> gate psum tile
            pT = {}       # (layer, step) -> joint T tile

            def tview(layer, step, d):
                i = DI[d]
                return pT[(layer, step)][:, i * 5 * B2:(i + 1) * 5 * B2]

            def alloc_T(layer, step, first=False):
                tj = tpool.tile([H, 10 * B2], SDT, name=f"t{layer}", tag=f"t{layer}")
                pT[(layer, step)] = tj
                if first:
                    nc.vector.memset(tj[:, 4 * B2:5 * B2], 0.0)
                    nc.vector.memset(tj[:, 9 * B2:10 * B2], 0.0)
                return tj

            def prep0(step, first=False):
                alloc_T(0, step, first)
                for d in "fb":
                    ps = psA if d == "f" else psB
                    g_t = ps.tile([H, 4 * B2], F32, name="g0" + d, tag="g" + d)
                    blk = sp0(d, step) * B2
                    for g in range(4):
                        nc.tensor.matmul(g_t[:, g * B2:(g + 1) * B2],
                                         wihT0_t[d][:, g * H:(g + 1) * H],
                                         xaug_t[:, blk:blk + B2],
                                         start=(g == 0), stop=False,
                                         skip_group_check=True)
                    pend[(0, d, step)] = g_t

            def prep1(step, first=False):
                alloc_T(1, step, first)
                for d in "fb":
                    ps = psA if d == "f" else psB
                    g_t = ps.tile([H, 4 * B2], F32, name="g1" + d, tag="g" + d)
                    blk = sp1(d, step) * 4 * B2
                    nc.tensor.matmul(g_t[:], idn_t[:],
                                     xg1[d][:, blk:blk + 4 * B2],
                                     start=True, stop=False, skip_group_check=True)
                    pend[(1, d, step)] = g_t

            def super_step(layer, whh, yt, sp, step, span):
                ctx = tc.high_priority(offset=150)
                ctx.__enter__()
                hp = {}
                for d in "fb":
                    if step == 0:
                        hp[d] = h00[:]
                    else:
                        pb = sp(d, step - 1) * B2
                        hp[d] = yt[d][:, pb:pb + B2]
                for d in "fb":
                    g_t = pend[(layer, d, step)]
                    for g in range(4):
                        nc.tensor.matmul(g_t[:, g * B2:(g + 1) * B2],
                                         whh[d][:, g * H:(g + 1) * H], hp[d],
                                         start=False, stop=True,
                                         skip_group_check=True)
                for d in "fb":
                    g_t = pend.pop((layer, d, step))
                    nc.scalar.activation(tview(layer, step, d)[:, 0:4 * B2],
                                         g_t[:], ACT_TANH)
                scr = {}
                for d in "fb":
                    t_t = tview(layer, step, d)
                    scr[d] = vpool.tile([H, 2 * B2], SDT, name=f"s{layer}{d}", tag=f"s{layer}{d}")
                    # scr = [(1+ti)*tg | (1+tf)*C]
                    nc.vector.scalar_tensor_tensor(scr[d][:], t_t[:, B2:3 * B2], 1.0,
                                                   t_t[:, 3 * B2:5 * B2], AluOp.add, AluOp.mult)
                for d in "fb":
                    nxt = tview(layer, step + 1, d)
                    nc.vector.scalar_tensor_tensor(nxt[:, 4 * B2:5 * B2], scr[d][:, B2:2 * B2],
                                                   0.5, scr[d][:, 0:B2], AluOp.mult, AluOp.add)
                # per-dir tanh(c): a joint instruction would lockstep the two
                # chains and destroy their engine-pipelining stagger (measured
                # +70% slot time) — keep the chains fully independent
                tc_t = {}
                for d in "fb":
                    nxt = tview(layer, step + 1, d)
                    tc_t[d] = vpool.tile([H, B2], SDT, name=f"c{layer}{d}", tag=f"c{layer}{d}")
                    nc.scalar.activation(tc_t[d][:], nxt[:, 4 * B2:5 * B2], ACT_TANH, scale=0.5)
                for d in "fb":
                    t_t = tview(layer, step, d)
                    ycol = sp(d, step) * B2
                    nc.vector.scalar_tensor_tensor(yt[d][:, ycol:ycol + B2], t_t[:, 0:B2],
                                                   1.0, tc_t[d][:], AluOp.add, AluOp.mult)
                ctx.__exit__(None, None, None)

            # ---------------- L1 projection chunks ----------------
            # chunk (d, c): span-slots [4c, 4c+4) of chain d; source y0 span
            # range offset: L1F slot s <- L0 span s; L1B slot j <- L0 span j+WARM
            # Bias folded into the scatter (per-partition per-gate scalar add);
            # the ctl/padkill matmul is only needed for the edge chunks whose
            # spans can contain out-of-range tokens.
            NPCH = S1 // 4
            src_off = {"f": 0, "b": WARM}
            EDGE = {("f", 0), ("f", 1), ("b", NPCH - 2), ("b", NPCH - 1)}

            def proj_chunk(d, c):
                s0 = 4 * c
                ycol = (src_off[d] + s0) * B2
                base = s0 * 4 * B2
                xv = xg1[d][:, base:base + 4 * 4 * B2].rearrange("h (sl c) -> h sl c", sl=4)
                for g in range(4):
                    p = psP.tile([H, 4 * B2], F32, name="pp", tag="pp")
                    nc.tensor.matmul(p[:], wih1Ta_t[d][:, g * H:(g + 1) * H],
                                     y0["f"][:, ycol:ycol + 4 * B2], start=True, stop=False)
                    nc.tensor.matmul(p[:], wih1Tb_t[d][:, g * H:(g + 1) * H],
                                     y0["b"][:, ycol:ycol + 4 * B2],
                                     start=False, stop=(d, c) not in EDGE)
                    if (d, c) in EDGE:
                        nc.tensor.matmul(p[:], ctlT1_t[d][:, g * H:(g + 1) * H],
                                         ctl_t[:, ycol:ycol + 4 * B2],
                                         start=False, stop=True)
                    # scatter + bias add; split across DVE and ACT
                    dst = xv[:, :, g * B2:(g + 1) * B2]
                    src = p[:].rearrange("h (sl ub) -> h sl ub", sl=4)
                    if g < 2:
                        nc.vector.tensor_scalar_add(dst, src, biasg_t[d][:, g:g + 1])
                    else:
                        nc.scalar.add(dst, src, biasg_t[d][:, g:g + 1])

            # ---------------- layer 0 ----------------
            prep0(0, first=True)
            prep0(1)
            for step in range(S0):
                if step + 2 < S0:
                    prep0(step + 2)
                if step == S0 - 1:
                    alloc_T(0, S0)
                super_step(0, whhT0_t, y0, sp0, step, S0)

            # ---------------- classifier chunks ----------------
            # chunk w (window span-slots [w, w+4)): y1F spans [w+WARM, w+WARM+4),
            # y1B spans [w, w+4); ready when both chains produced them.
            h1pool = clssb

            def cls_chunk(w):
                CH = 4 * B2
                fcol = (w + WARM) * B2
                bcol = w * B2
                h1 = [h1pool.tile([H, CH], BF16, name="h1a", tag="h1a"),
                      h1pool.tile([H, CH], BF16, name="h1b", tag="h1b")]
                for m in range(2):
                    p = psP.tile([H, CH], F32, name="pc", tag="pp")
                    nc.tensor.matmul(p[:], w1Ta_t[:, m * H:(m + 1) * H],
                                     y1["f"][:, fcol:fcol + CH], start=True, stop=False)
                    nc.tensor.matmul(p[:], w1Tb_t[:, m * H:(m + 1) * H],
                                     y1["b"][:, bcol:bcol + CH], start=False, stop=True)
                    nc.scalar.activation(h1[m][:], p[:], ACT_RELU,
                                         bias=b1col_t[:, m:m + 1])
                # final GEMM transposed: out[row, d], row = span*B2 + u*B + b
                p = psP.tile([H, 4 * D], F32, name="po", tag="pp")
                for j in range(4):
                    cj = j * B2
                    nc.tensor.matmul(p[:, j * D:(j + 1) * D], h1[0][:, cj:cj + B2],
                                     w2Ta_t[:], start=True, stop=False)
                    nc.tensor.matmul(p[:, j * D:(j + 1) * D], h1[1][:, cj:cj + B2],
                                     w2Tb_t[:], start=False, stop=False)
                    nc.tensor.matmul(p[:, j * D:(j + 1) * D],
                                     ctl_t[0:1, fcol + cj:fcol + cj + B2],
                                     b2row_t[:], start=False, stop=True)
                o_t = clssb.tile([H, 4 * D], F32, name="ot", tag="ot")
                nc.scalar.activation(o_t[:], p[:], ACT_TANH)
                # one DMA for all 4 span-blocks (1KB descriptors instead of 256B)
                ov = out[w * B2:(w + 4) * B2, :].rearrange("(j p) c -> p j c", p=B2)
                nc.sync.dma_start(out=ov, in_=o_t[:].rearrange("h (j c) -> h j c", j=4))

            cls_ready = {}
            for w in range(0, SW, 4):
                r = max(w + WARM + 3, S1 - 1 - w)
                cls_ready.setdefault(r, []).append(w)

            # ---------------- dense projection burst ----------------
            # All proj chunks back-to-back between the loops: the sustained
            # matmul stream un-throttles the PE HAM clock gate (1.2 -> 2.4
            # GHz), and the scatters overlap on DVE/ACT.  Ordered by L1
            # consumption (f ascending, b descending, interleaved).
            for c in range(NPCH):
                proj_chunk("f", c)
                proj_chunk("b", NPCH - 1 - c)

            # ---------------- layer 1 ----------------
            prep1(0, first=True)
            prep1(1)
            for step in range(S1):
                if step + 2 < S1:
                    prep1(step + 2)
                if step == S1 - 1:
                    alloc_T(1, S1)
                super_step(1, whhT1_t, y1, sp1, step, S1)
                for w in cls_ready.get(step, ()):
                    cls_chunk(w)

    nc.compile()
    return nc


# ======================= host side =======================

def _prep_weights(inp):
    """Returns dict of np arrays shared by all cores (bf16).

    Gate row-blocks reordered from reference [i,f,g,o] to device [o,i,f,g];
    i,f,o rows scaled 0.5 (one-tanh trick)."""
    H_ = H
    sr = np.full((4 * H_, 1), 0.5, np.float32)
    sr[2 * H_:3 * H_] = 1.0

    def reorder(a):           # rows [i,f,g,o] -> [o,i,f,g]
        return np.concatenate([a[3 * H_:], a[:H_], a[H_:2 * H_], a[2 * H_:3 * H_]], 0)

    w = {}
    for d, tag in (("f", "0"), ("b", "1")):
        Wih, Whh = inp[f"Wih0{tag}"], inp[f"Whh0{tag}"]
        bias = inp[f"bih0{tag}"] + inp[f"bhh0{tag}"]
        wihT = reorder(np.concatenate([Wih * sr, (bias[:, None] * sr)], 1)).T  # [65, 4H]
        w[f"wihT0{d}"] = np.concatenate(
            [wihT, np.zeros((KP - D - 1, 4 * H_), np.float32)], 0).astype(bf16)
        w[f"whhT0{d}"] = reorder(Whh * sr * 0.5).T.astype(bf16)
        Wih1, Whh1 = inp[f"Wih1{tag}"], inp[f"Whh1{tag}"]
        bias1 = reorder((inp[f"bih1{tag}"] + inp[f"bhh1{tag}"])[:, None] * sr)  # [4H,1]
        w[f"whhT1{d}"] = reorder(Whh1 * sr * 0.5).T.astype(bf16)
        w[f"wih1Ta{d}"] = reorder(Wih1[:, :H] * sr * 0.5).T.astype(bf16)
        w[f"wih1Tb{d}"] = reorder(Wih1[:, H:] * sr * 0.5).T.astype(bf16)
        # L1 bias applied during the scatter: [H, 4] per gate column
        w[f"biasg{d}"] = bias1.reshape(4, H).T.astype(np.float32)
        padkill = np.zeros((1, 4 * H), np.float32)
        padkill[0, H:2 * H] = PADKILL      # i-gate block (device order [o,i,f,g])
        w[f"ctlT1{d}"] = np.concatenate([np.zeros((1, 4 * H), np.float32),
                                         padkill], 0).astype(bf16)
    w["idn"] = np.eye(H, dtype=np.float32).astype(bf16)
    w["w1Ta"] = (0.5 * inp["W1"][:, :H]).T.astype(bf16)
    w["w1Tb"] = (0.5 * inp["W1"][:, H:]).T.astype(bf16)
    w["b1col"] = inp["b1"].reshape(2, H).T.astype(np.float32)
    w["w2Ta"] = inp["W2"][:, :H].T.astype(bf16)
    w["w2Tb"] = inp["W2"][:, H:].T.astype(bf16)
    w["b2row"] = inp["b2"][None, :].astype(bf16)
    return w


def _per_core_inputs(x, q):
    """x: [B, T, D] f32.  Builds xaug [KP, S0*B2] and ctl [2, S0*B2] in
    span-slot layout: col = s*B2 + u*B + b, token = 64q + SW*u + s - WARM."""
    xaug = np.zeros((KP, S0 * B2), np.float32)
    ctl = np.zeros((2, S0 * B2), np.float32)
    for s in range(S0):
        for u in range(NU):
            t = WIN * q + SW * u + s - WARM
            sl = slice(s * B2 + u * B, s * B2 + (u + 1) * B)
            if 0 <= t < T:
                xaug[:D, sl] = x[:, t, :].T
                xaug[D, sl] = 1.0
                ctl[0, sl] = 1.0
            else:
                ctl[1, sl] = 1.0
    return xaug.astype(bf16), ctl.astype(bf16)


def _get_program():
    if "nc" not in _CACHE:
        _CACHE["nc"] = _build_program()
    return _CACHE["nc"]


def _run(inputs, trace=False):
    inp = {k: np.asarray(v) for k, v in inputs.items()}
    nc = _get_program()
    w = _prep_weights(inp)
    x = inp["x"].astype(np.float32)
    in_maps = []
    for q in range(NC):
        xaug, ctl = _per_core_inputs(x, q)
        m = dict(w)
        m["xaug"] = xaug
        m["ctl"] = ctl
        in_maps.append(m)
    res = run_bass_kernel_spmd(nc, in_maps, list(range(NC)), trace=trace)
    outp = np.zeros((B, T, D), np.float32)
    for q in range(NC):
        o = res.results[q]["out"].reshape(SW, NU, B, D)   # [span, u, b, d]
        for u in range(NU):
            outp[:, WIN * q + SW * u:WIN * q + SW * (u + 1), :] = \
                o[:, u].transpose(1, 0, 2)
    return outp, res


def kernel(**inputs):
    out, _ = _run(inputs, trace=False)
    return out
